# revision 39
# baseline (speedup 1.0000x reference)
"""Trainium2 Bass kernel for a 2-layer causal transformer LM (B=2, L=1024,
D=768, H=12, FF=3072, V=32000) with box-sparse attention mask.

Sharding over 8 NeuronCores: 2-way data parallel over batch x 4-way tensor
parallel within each batch group:
  - attention: 3 heads/core, full-L keys/queries
  - three per-head AllGathers per layer turn head-sharding into
    sequence-sharding (each head's gather overlaps the next head's
    compute), then the out-projection and the full-width FFN run
    sequence-parallel on each core's L/4 position slice
  - the residual stream lives sequence-sharded [768, 256] f32; each layer
    ends with the NEXT LayerNorm (ln1 of l+1, or lnf) computed on the local
    slice and one bf16 AllGather of the post-LN activations; the LN
    statistics matmuls are interleaved into the preceding out-proj / FFN-w2
    loops so the gather triggers as early as possible
  - LM head: V/4 vocab slice per core, bf16 weights streamed in quarters

Device layout: activations transposed [feature, position]; matmuls in
bf16 with f32 PSUM accumulation; LN stats via ones-matmuls on the PE;
softmax without max subtraction (scores are O(1)); the box mask is applied
as a post-exp 0/1 multiply on the vector engine (only for partially-masked
blocks; fully-allowed blocks skip it); softmax denominator via an appended
ones-column in the A@V matmul, broadcast back over partitions with a PE
ones-matmul.
"""
import sys

sys.path.insert(0, "/opt/trn_rl_repo")

from contextlib import ExitStack

import numpy as np
import concourse.bass as bass
import concourse.bacc as bacc
import concourse.mybir as mybir
import concourse.tile as tile
from concourse.bass_utils import run_bass_kernel_spmd

F32 = mybir.dt.float32
F32R = mybir.dt.float32r
BF16 = mybir.dt.bfloat16
AF = mybir.ActivationFunctionType
ALU = mybir.AluOpType

B, L, D, H, HD = 2, 1024, 768, 12, 64
FF, V, NL = 3072, 32000, 2
BOS, SEP, WIN = 1, 2, 512
EPS = 1e-5
TP = 4                      # tensor-parallel group size
NH = H // TP                # heads per core (3)
LS = L // TP                # sequence slice per core (256)
VS = V // TP                # vocab slice per core (8000)
KT = D // 128               # k-tiles over model dim (6)
FT = FF // 128              # k-tiles over ff dim (24)
IT = L // 128               # i/j tiles over positions (8)
IC = L // 512               # 512-wide position chunks (2)
GELU_FUNC = None            # sim-only override hook (AF.Gelu on hardware)


def _mask_allowed(tokens, attn_mask):
    """(B, L, L) boolean allowed[i, j] per reference._box_mask_bias."""
    valid = attn_mask.astype(bool)
    ii = np.arange(L)[:, None]
    jj = np.arange(L)[None, :]
    causal = jj <= ii
    is_sep = (tokens == SEP) & valid
    seg = np.cumsum(is_sep.astype(np.int32), axis=1)
    same_seg = seg[:, :, None] == seg[:, None, :]
    gkey = ((tokens == BOS) & valid) | is_sep
    win = (ii - jj) <= WIN
    return valid[:, None, :] & causal[None] & (
        same_seg | gkey[:, None, :] | win[None])


def _sbufify(w, dtype=np.float32):
    """(K, M) host matrix -> [128, (K/128)*M] SBUF layout; k-tile kt at
    columns [kt*M:(kt+1)*M)."""
    K, M = w.shape
    assert K % 128 == 0
    return np.ascontiguousarray(
        w.reshape(K // 128, 128, M).transpose(1, 0, 2)
        .reshape(128, (K // 128) * M)).astype(dtype)


def _build(live, av_live, use_gelu_bias, partial, trims):
    nc = bacc.Bacc("TRN2", target_bir_lowering=False)

    hln0t = nc.declare_dram_parameter("hln0t", [128, KT * L], BF16,
                                      isOutput=False)
    x0s = nc.declare_dram_parameter("x0s", [128, KT * LS], F32R,
                                    isOutput=False)
    maskt = nc.declare_dram_parameter("maskt", [128, IT * L], BF16,
                                      isOutput=False)
    wqk = nc.declare_dram_parameter("wqk", [128, NL * KT * 384], BF16,
                                    isOutput=False)
    wv = nc.declare_dram_parameter("wv", [128, NL * KT * 192], BF16,
                                   isOutput=False)
    wo = nc.declare_dram_parameter("wo", [128, NL * KT * 768], BF16,
                                   isOutput=False)
    w1 = nc.declare_dram_parameter("w1", [128, NL * FT * (KT * 128)], BF16,
                                   isOutput=False)
    gb = nc.declare_dram_parameter("gb", [128, NL * FT], F32, isOutput=False)
    w2 = nc.declare_dram_parameter("w2", [128, NL * KT * (FT * 128)], BF16,
                                   isOutput=False)
    wh = nc.declare_dram_parameter("wh", [128, KT * VS], BF16, isOutput=False)
    logits = nc.declare_dram_parameter("logits", [L, VS], BF16, isOutput=True)

    groups = [[0, 1, 2, 3], [4, 5, 6, 7]]

    nvc = (VS + 511) // 512
    vchunks = [(i * 512, min(512, VS - i * 512)) for i in range(nvc)]
    quarters = [vchunks[i:i + 4] for i in range(0, nvc, 4)]

    with tile.TileContext(nc) as tc, ExitStack() as ctx:
        const = ctx.enter_context(tc.tile_pool(name="const", bufs=1))
        dram = ctx.enter_context(tc.tile_pool(name="dram", bufs=1,
                                              space="DRAM"))
        resb = ctx.enter_context(tc.tile_pool(name="resb", bufs=1))
        rows = ctx.enter_context(tc.tile_pool(name="rows", bufs=1))
        hw = ctx.enter_context(tc.tile_pool(name="hw", bufs=2))

        ones_col = const.tile([128, 1], F32R, name="ones")
        nc.gpsimd.memset(ones_col[:].bitcast(F32), 1.0)
        eps_col = const.tile([1, 1], F32, name="epsc")
        nc.gpsimd.memset(eps_col[:], EPS)
        ones_row = const.tile([1, 128], F32R, name="onesr")
        nc.gpsimd.memset(ones_row[:].bitcast(F32), 1.0)
        seq_off = (nc.partition_id() % TP) * LS

        # persistent state: mask tiles (partial blocks only), post-LN
        # activations, residual slice, and all attention weights (loaded
        # once at startup so no weight DMA ever contends with the
        # latency-critical collectives)
        mtiles = {pc: resb.tile([128, 512], BF16,
                                name=f"msk_{pc[0]}_{pc[1]}")
                  for pc in partial}
        hln = [resb.tile([128, L], BF16, name=f"hln_{k}") for k in range(KT)]
        xs = [resb.tile([128, LS], F32R, name=f"xs_{k}") for k in range(KT)]
        wqk_sb = resb.tile([128, NL * KT * 384], BF16, name="wqkA")
        wv_sb = resb.tile([128, NL * KT * 192], BF16, name="wvA")
        wo_sb = resb.tile([128, NL * KT * 768], BF16, name="woA")

        def load_quarter(vq, gate=None):
            chunks = quarters[vq]
            q0 = chunks[0][0]
            qw = sum(w for _, w in chunks)
            tiles = []
            for k in range(KT):
                t = hw.tile([128, 2000], BF16, name=f"hw_{k}")
                if gate is not None:
                    nc.sync.dma_start(t[0:1, 0:1], gate[0:1, 0:1])
                nc.sync.dma_start(t[:, 0:qw],
                                  wh[:, k * VS + q0:k * VS + q0 + qw])
                tiles.append(t)
            return tiles

        wt_cur = None

        with ExitStack() as body:
            hpool = body.enter_context(tc.tile_pool(name="hpool", bufs=1))
            pm = body.enter_context(tc.tile_pool(name="pm", bufs=6,
                                                 space="PSUM"))
            py = body.enter_context(tc.tile_pool(name="py", bufs=2,
                                                 space="PSUM"))

            def ln_stat(k, src, sx, sxx):
                """Accumulate per-position sum/sumsq of k-tile `src` into
                PSUM rows sx/sxx (start at k==0, stop at k==KT-1)."""
                nc.tensor.matmul(sx[0:1, 0:LS], ones_col[:], src[:, 0:LS],
                                 start=(k == 0), stop=(k == KT - 1))
                xx = hpool.tile([128, LS], F32R, name="lnxx", bufs=2)
                nc.scalar.activation(xx[:, 0:LS], src[:, 0:LS], AF.Square)
                nc.tensor.matmul(sxx[0:1, 0:LS], ones_col[:], xx[:, 0:LS],
                                 start=(k == 0), stop=(k == KT - 1))

            def ln_rows(sx, sxx):
                """Turn the PSUM stats rows into broadcast [128, LS] PSUM
                tiles (rstd, rho=mu*rstd)."""
                mu = rows.tile([1, 512], F32, name="mu")
                nc.vector.tensor_scalar_mul(mu[0:1, 0:LS], sx[0:1, 0:LS],
                                            1.0 / D)
                sq = rows.tile([1, 512], F32, name="sq")
                nc.vector.tensor_mul(sq[0:1, 0:LS], mu[0:1, 0:LS],
                                     mu[0:1, 0:LS])
                dvar = rows.tile([1, 512], F32, name="dvar")
                nc.vector.scalar_tensor_tensor(
                    dvar[0:1, 0:LS], sq[0:1, 0:LS], -float(D),
                    sxx[0:1, 0:LS], op0=ALU.mult, op1=ALU.add)
                rstd = rows.tile([1, 512], F32, name="rstd")
                nc.scalar.activation(rstd[0:1, 0:LS], dvar[0:1, 0:LS],
                                     AF.Sqrt, bias=eps_col[:],
                                     scale=1.0 / D)
                rtmp = rows.tile([1, 512], F32, name="rtmp")
                nc.vector.reciprocal_approx_fast(rtmp[0:1, 0:LS],
                                                 rstd[0:1, 0:LS])
                rstd_row = rows.tile([1, 512], F32R, name="rstd_row")
                with nc.allow_low_precision(reason="f32r feeds bcast"):
                    nc.vector.tensor_copy(rstd_row[0:1, 0:LS],
                                          rtmp[0:1, 0:LS])
                rho_row = rows.tile([1, 512], F32R, name="rho_row")
                nc.vector.tensor_mul(rho_row[0:1, 0:LS], mu[0:1, 0:LS],
                                     rtmp[0:1, 0:LS])
                prs = pm.tile([128, 512], F32, name="pmm")
                nc.tensor.matmul(prs[:, 0:LS], ones_row[:],
                                 rstd_row[0:1, 0:LS], start=True, stop=True)
                pro = pm.tile([128, 512], F32, name="pmm")
                nc.tensor.matmul(pro[:, 0:LS], ones_row[:],
                                 rho_row[0:1, 0:LS], start=True, stop=True)
                return prs, pro

            def ln_apply(src, dst, bcs, c0=0, w=LS):
                prs, pro = bcs
                nc.vector.scalar_tensor_tensor(
                    dst[:, 0:w], src[:, c0:c0 + w], 1.0, prs[:, c0:c0 + w],
                    op0=ALU.bypass, op1=ALU.mult)
                nc.vector.tensor_sub(dst[:, 0:w], dst[:, 0:w],
                                     pro[:, c0:c0 + w])

            # startup loads in priority order: layer-0 QKV weights +
            # layer-0 LN1 (host-precomputed) gate the first matmuls;
            # everything else streams in behind them
            nc.sync.dma_start(wqk_sb[:, 0:KT * 384], wqk[:, 0:KT * 384])
            for k in range(KT):
                nc.sync.dma_start(hln[k][:], hln0t[:, k * L:(k + 1) * L])
            nc.sync.dma_start(wv_sb[:, 0:KT * 192], wv[:, 0:KT * 192])
            nc.sync.dma_start(wqk_sb[:, KT * 384:], wqk[:, KT * 384:])
            nc.sync.dma_start(wv_sb[:, KT * 192:], wv[:, KT * 192:])
            nc.sync.dma_start(wo_sb[:], wo[:, :])
            for k in range(KT):
                nc.sync.dma_start(xs[k][:], x0s[:, k * LS:(k + 1) * LS])
            for (jt, c) in partial:
                nc.sync.dma_start(
                    mtiles[(jt, c)][:],
                    maskt[:, jt * L + c * 512:jt * L + c * 512 + 512])

            for l in range(NL):
                with ExitStack() as lay:
                    wpf = lay.enter_context(tc.tile_pool(name="wpf", bufs=1))
                    W1PF, W2PF = 10, 3

                    def load_w1(mt, gate=None):
                        t = wpf.tile([128, KT * 128], BF16, name="w1c",
                                     bufs=W1PF)
                        if gate is not None:
                            # dummy write creates a WAW dep: the real load
                            # must wait for the gather, so the prefetch
                            # burst cannot steal collective bandwidth
                            nc.sync.dma_start(t[0:1, 0:1], gate[0:1, 0:1])
                        nc.sync.dma_start(
                            t[:], w1[:, (l * FT + mt) * 768:
                                      (l * FT + mt + 1) * 768])
                        return t

                    def load_w2(mt, gate=None):
                        t = wpf.tile([128, FT * 128], BF16, name="w2c",
                                     bufs=W2PF)
                        if gate is not None:
                            nc.sync.dma_start(t[0:1, 0:1], gate[0:1, 0:1])
                        nc.sync.dma_start(
                            t[:], w2[:, (l * KT + mt) * FT * 128:
                                      (l * KT + mt + 1) * FT * 128])
                        return t

                    wqo = l * KT * 384
                    wvo = l * KT * 192
                    woo = l * KT * 768

                    # ================ attention ================
                    with ExitStack() as attn:
                        qkv = attn.enter_context(
                            tc.tile_pool(name="qkv", bufs=1))
                        qp = [qkv.tile([64, L], BF16, name=f"qp{h}")
                              for h in range(NH)]
                        kp = [qkv.tile([64, L], BF16, name=f"kp{h}")
                              for h in range(NH)]
                        vt = [qkv.tile([128, 3 * 65], BF16, name=f"v{j}")
                              for j in range(IT)]
                        yh = [qkv.tile([64, L], BF16, name=f"yh{h}")
                              for h in range(NH)]
                        for j in range(IT):
                            for h in range(NH):
                                nc.gpsimd.memset(
                                    vt[j][:, h * 65 + 64:h * 65 + 65], 1.0)

                        with ExitStack() as s1:
                            # pre-load the exp activation table while the
                            # QKV matmuls run (a table switch costs ~1.3us
                            # on the scalar engine)
                            wrm = rows.tile([1, 1], F32, name="wrm")
                            nc.scalar.activation(wrm[:], eps_col[:], AF.Exp)
                            qk_dest = [(qp[0], qp[1]), (qp[2], kp[0]),
                                       (kp[1], kp[2])]
                            for mt in range(3):
                                for c in range(IC):
                                    p = pm.tile([128, 512], F32, name="pmm")
                                    for k in range(KT):
                                        o = wqo + k * 384 + mt * 128
                                        nc.tensor.matmul(
                                            p[:],
                                            wqk_sb[:, o:o + 128],
                                            hln[k][:, c * 512:(c + 1) * 512],
                                            start=(k == 0),
                                            stop=(k == KT - 1))
                                    t0, t1 = qk_dest[mt]
                                    cs = slice(c * 512, (c + 1) * 512)
                                    nc.vector.tensor_copy(t0[:, cs],
                                                          p[0:64, :])
                                    nc.scalar.activation(t1[:, cs],
                                                         p[64:128, :],
                                                         AF.Copy)
                            for j in range(IT):
                                p = pm.tile([128, 512], F32, name="pmm")
                                for k in range(KT):
                                    o = wvo + k * 192
                                    nc.tensor.matmul(
                                        p[:, 0:192],
                                        hln[k][:, j * 128:(j + 1) * 128],
                                        wv_sb[:, o:o + 192],
                                        start=(k == 0), stop=(k == KT - 1))
                                for h in range(NH):
                                    nc.vector.tensor_copy(
                                        vt[j][:, h * 65:h * 65 + 64],
                                        p[:, h * 64:(h + 1) * 64])

                        # one small AllGather per head: head h's gather is
                        # in flight while head h+1 computes, so only the
                        # last (384KB-recv) gather is exposed on the
                        # critical path.
                        agy_in = [dram.tile([64, L], BF16,
                                            name=f"agy_in{l}_{h}")
                                  for h in range(NH)]
                        agy_out = [dram.tile([TP * 64, L], BF16,
                                             name=f"agy_out{l}_{h}")
                                   for h in range(NH)]
                        with ExitStack() as s2:
                            epool = s2.enter_context(
                                tc.tile_pool(name="epool", bufs=24))
                            spool = s2.enter_context(
                                tc.tile_pool(name="spool", bufs=4))
                            live_co = sorted(live, key=lambda t: (t[1],
                                                                  t[0]))

                            def emit_scores(h):
                                et = {}
                                for (jt, c) in live_co:
                                    t0 = trims[(jt, c)]
                                    cs = slice(t0, 512)
                                    p = pm.tile([128, 512], F32, name="pmm")
                                    nc.tensor.matmul(
                                        p[:, cs],
                                        kp[h][:, jt * 128:(jt + 1) * 128],
                                        qp[h][:, c * 512 + t0:
                                              (c + 1) * 512],
                                        start=True, stop=True)
                                    e = epool.tile([128, 512], BF16,
                                                   name="e")
                                    nc.scalar.activation(e[:, cs], p[:, cs],
                                                         AF.Exp)
                                    if (jt, c) in partial:
                                        nc.vector.tensor_mul(
                                            e[:, cs], e[:, cs],
                                            mtiles[(jt, c)][:, cs])
                                    et[(jt, c)] = e
                                return et

                            def emit_av(h, et):
                                for c in range(IC):
                                    jts = av_live[c]
                                    p = py.tile([128, 512], F32, name="pyy")
                                    for n, jt in enumerate(jts):
                                        t0 = trims[(jt, c)]
                                        nc.tensor.matmul(
                                            p[0:65, t0:512],
                                            vt[jt][:, h * 65:h * 65 + 65],
                                            et[(jt, c)][:, t0:512],
                                            start=(n == 0),
                                            stop=(n == len(jts) - 1),
                                            skip_group_check=True)
                                    den = rows.tile([1, 512], F32,
                                                    name="den")
                                    nc.vector.tensor_copy(den[:],
                                                          p[64:65, :])
                                    rtm = rows.tile([1, 512], F32,
                                                    name="rtm")
                                    nc.vector.reciprocal_approx_fast(
                                        rtm[:], den[:])
                                    rec = rows.tile([1, 512], F32R,
                                                    name="rec")
                                    with nc.allow_low_precision(
                                            reason="softmax denom"):
                                        nc.vector.tensor_copy(rec[:],
                                                              rtm[:])
                                    pb = py.tile([128, 512], F32,
                                                 name="pyy")
                                    nc.tensor.matmul(
                                        pb[0:64, :], ones_row[:, 0:64],
                                        rec[:], start=True, stop=True)
                                    rb = spool.tile([64, 512], BF16,
                                                    name="rb", bufs=2)
                                    # DVE copy: keep the scalar engine free
                                    # for the exp chain
                                    nc.vector.tensor_copy(rb[:],
                                                          pb[0:64, :])
                                    nc.vector.tensor_mul(
                                        yh[h][:, c * 512:(c + 1) * 512],
                                        p[0:64, :], rb[:])
                                # gather this head's y as soon as it is
                                # finished
                                nc.sync.dma_start(agy_in[h][:], yh[h][:])
                                nc.gpsimd.collective_compute(
                                    "AllGather", ALU.bypass,
                                    replica_groups=groups,
                                    ins=[agy_in[h].opt()],
                                    outs=[agy_out[h].opt()])

                            # software-pipelined: the PE computes head
                            # h+1's scores while the scalar engine is
                            # still exponentiating head h's
                            ets = {0: emit_scores(0)}
                            for h in range(NH):
                                if h + 1 < NH:
                                    ets[h + 1] = emit_scores(h + 1)
                                emit_av(h, ets.pop(h))

                        # sequence-parallel out-projection.  Gathered
                        # feature f = 192*s + 64*t + w64 lives at
                        # agy_out[t] row s*64 + w64.
                        with ExitStack() as s3:
                            wpo = s3.enter_context(
                                tc.tile_pool(name="wpo", bufs=1))
                            yrt = [wpo.tile([128, LS], BF16, name=f"yrt_{k}")
                                   for k in range(KT)]

                            f = 0
                            while f < 768:
                                s = f // 192
                                w = f - 192 * s
                                t, w64 = w // 64, w % 64
                                n = min(64 - w64, 128 - f % 128)
                                nc.sync.dma_start(
                                    yrt[f // 128][f % 128:f % 128 + n, :],
                                    agy_out[t][s * 64 + w64:
                                               s * 64 + w64 + n,
                                               bass.ds(seq_off, LS)])
                                f += n
                            # FFN / LM-head weight prefetch is gated on
                            # the last y-gather: an ungated burst steals
                            # HBM bandwidth from the collectives (measured
                            # 15-22GB/s collective bus during the burst)
                            gate = agy_out[NH - 1]
                            w1t = [load_w1(mt, gate) for mt in range(W1PF)]
                            w2t = [load_w2(mt, gate) for mt in range(W2PF)]
                            gb_sb = None
                            if use_gelu_bias[l]:
                                gb_sb = wpf.tile([128, FT], F32, name="gb")
                                nc.sync.dma_start(
                                    gb_sb[:], gb[:, l * FT:(l + 1) * FT])
                            if l == NL - 1:
                                wt_cur = load_quarter(0, gate)
                            # pre-load the sqrt table for ln2's ln_rows
                            wrm2 = rows.tile([1, 1], F32, name="wrm2")
                            nc.scalar.activation(wrm2[:], eps_col[:],
                                                 AF.Sqrt)
                            sx2 = py.tile([128, 512], F32, name="pyy")
                            sxx2 = py.tile([128, 512], F32, name="pyy")
                            for mt in range(KT):
                                p = pm.tile([128, 512], F32, name="pmm")
                                for k in range(KT):
                                    o = woo + k * 768 + mt * 128
                                    nc.tensor.matmul(
                                        p[:, 0:LS],
                                        wo_sb[:, o:o + 128],
                                        yrt[k][:],
                                        start=(k == 0), stop=(k == KT - 1))
                                nc.vector.tensor_add(
                                    xs[mt][:], xs[mt][:], p[:, 0:LS])
                                ln_stat(mt, xs[mt], sx2, sxx2)

                    # ============ FFN (sequence-parallel) ============
                    with ExitStack() as ffn:
                        mpool = ffn.enter_context(
                            tc.tile_pool(name="mpool", bufs=1))
                        bcs2 = ln_rows(sx2, sxx2)
                        h2s = []
                        for k in range(KT):
                            h = hpool.tile([128, LS], BF16, name=f"h2_{k}")
                            ln_apply(xs[k], h, bcs2)
                            h2s.append(h)
                        mtl = []
                        for mt in range(FT):
                            if mt + W1PF < FT:
                                w1t.append(load_w1(mt + W1PF))
                            p = pm.tile([128, 512], F32, name="pmm")
                            for k in range(KT):
                                nc.tensor.matmul(
                                    p[:, 0:LS],
                                    w1t[mt][:, k * 128:(k + 1) * 128],
                                    h2s[k][:],
                                    start=(k == 0), stop=(k == KT - 1))
                            m = mpool.tile([128, LS], BF16, name=f"m_{mt}")
                            gf = GELU_FUNC or AF.Gelu
                            if gb_sb is not None:
                                nc.scalar.activation(
                                    m[:], p[:, 0:LS], gf,
                                    bias=gb_sb[:, mt:mt + 1])
                            else:
                                nc.scalar.activation(m[:], p[:, 0:LS], gf)
                            mtl.append(m)
                        # pre-load the sqrt table for the boundary LN so
                        # the table switch is off the critical chain
                        wrm3 = rows.tile([1, 1], F32, name="wrm3")
                        nc.scalar.activation(wrm3[:], eps_col[:], AF.Sqrt)
                        sx3 = py.tile([128, 512], F32, name="pyy")
                        sxx3 = py.tile([128, 512], F32, name="pyy")
                        for mt in range(KT):
                            if mt + W2PF < KT:
                                w2t.append(load_w2(mt + W2PF))
                            p = pm.tile([128, 512], F32, name="pmm")
                            for k in range(FT):
                                nc.tensor.matmul(
                                    p[:, 0:LS],
                                    w2t[mt][:, k * 128:(k + 1) * 128],
                                    mtl[k][:],
                                    start=(k == 0), stop=(k == FT - 1))
                            nc.vector.tensor_add(xs[mt][:], xs[mt][:],
                                                 p[:, 0:LS])
                            ln_stat(mt, xs[mt], sx3, sxx3)

                    # ===== next LN on the local slice + AllGather =====
                    with ExitStack() as nxs:
                        npool = nxs.enter_context(
                            tc.tile_pool(name="npool", bufs=1))
                        bcs3 = ln_rows(sx3, sxx3)
                        if l < NL - 1:
                            # two k-half gathers: QKV k-tile accumulation
                            # starts on the first half while the second is
                            # still in flight
                            KH = KT // 2
                            for hf in range(2):
                                agh_in = dram.tile([KH * 128, LS], BF16,
                                                   name=f"agh_in{l}_{hf}")
                                agh_out = dram.tile([TP * KH * 128, LS],
                                                    BF16,
                                                    name=f"agh_out{l}_{hf}")
                                for k in range(KH):
                                    kk = hf * KH + k
                                    nxt = npool.tile([128, LS], BF16,
                                                     name=f"nx_{kk}")
                                    ln_apply(xs[kk], nxt, bcs3)
                                    nc.sync.dma_start(
                                        agh_in[k * 128:(k + 1) * 128, :],
                                        nxt[:])
                                nc.gpsimd.collective_compute(
                                    "AllGather", ALU.bypass,
                                    replica_groups=groups,
                                    ins=[agh_in.opt()],
                                    outs=[agh_out.opt()])
                                # k-major so hln[k] completes after 4 DMAs
                                for k in range(KH):
                                    for q in range(TP):
                                        nc.sync.dma_start(
                                            hln[hf * KH + k][
                                                :, q * LS:(q + 1) * LS],
                                            agh_out[q * KH * 128 + k * 128:
                                                    q * KH * 128 +
                                                    (k + 1) * 128, :])
                        else:
                            # final boundary: two position-half gathers;
                            # the LM head consumes the first half's
                            # i-tiles while the second is in flight
                            for hf in range(2):
                                agh_in = dram.tile([KT * 128, 128], BF16,
                                                   name=f"agh_in{l}_{hf}")
                                agh_out = dram.tile([TP * KT * 128, 128],
                                                    BF16,
                                                    name=f"agh_out{l}_{hf}")
                                for k in range(KT):
                                    nxt = npool.tile([128, 128], BF16,
                                                     name=f"nxp_{hf}_{k}")
                                    ln_apply(xs[k], nxt, bcs3,
                                             c0=hf * 128, w=128)
                                    nc.sync.dma_start(
                                        agh_in[k * 128:(k + 1) * 128, :],
                                        nxt[:])
                                nc.gpsimd.collective_compute(
                                    "AllGather", ALU.bypass,
                                    replica_groups=groups,
                                    ins=[agh_in.opt()],
                                    outs=[agh_out.opt()])
                                for k in range(KT):
                                    for q in range(TP):
                                        nc.sync.dma_start(
                                            hln[k][:,
                                                   q * LS + hf * 128:
                                                   q * LS + hf * 128
                                                   + 128],
                                            agh_out[q * KT * 128 + k * 128:
                                                    q * KT * 128 +
                                                    (k + 1) * 128, :])

        # ================ LM head ================
        # hln now holds lnf(x) over the full sequence, bf16.
        with ExitStack() as headx:
            ob = headx.enter_context(tc.tile_pool(name="ob", bufs=4))
            ph = headx.enter_context(tc.tile_pool(name="ph", bufs=8,
                                                  space="PSUM"))
            ci = 0
            for vq, chunks in enumerate(quarters):
                q0, qw = chunks[0][0], sum(w for _, w in chunks)
                wt = wt_cur
                if vq + 1 < len(quarters):
                    wt_cur = load_quarter(vq + 1)
                # even i-tiles arrive with the first position-half gather
                for it in (0, 2, 4, 6, 1, 3, 5, 7):
                    ps = [ph.tile([128, 512], F32, name="phh")
                          for _ in range(len(chunks))]
                    # k-outer so the stationary tile (hln it-slice) is
                    # identical for the 4 consecutive matmuls
                    for k in range(KT):
                        for vc, (v0, w) in enumerate(chunks):
                            nc.tensor.matmul(
                                ps[vc][:, 0:w],
                                hln[k][:, it * 128:(it + 1) * 128],
                                wt[k][:, v0 - q0:v0 - q0 + w],
                                start=(k == 0), stop=(k == KT - 1))
                    o = ob.tile([128, 2000], BF16, name="o")
                    for vc, (v0, w) in enumerate(chunks):
                        if ci % 2 == 0:
                            nc.vector.tensor_copy(o[:, v0 - q0:v0 - q0 + w],
                                                  ps[vc][:, 0:w])
                        else:
                            nc.scalar.activation(o[:, v0 - q0:v0 - q0 + w],
                                                 ps[vc][:, 0:w], AF.Copy)
                        ci += 1
                    nc.sync.dma_start(
                        logits[it * 128:(it + 1) * 128, q0:q0 + qw],
                        o[:, 0:qw])
    nc.finalize()
    return nc


_PROG_CACHE = {}


def _prepare(inputs):
    tokens = np.asarray(inputs["tokens"])
    types = np.asarray(inputs["types"])
    attn_mask = np.asarray(inputs["attn_mask"])
    f = {k: np.asarray(inputs[k], dtype=np.float32) for k in
         ("tok_emb", "type_emb", "pos_emb", "qkv_w", "out_w", "ln1_s",
          "ln1_b", "ln2_s", "ln2_b", "ff_w1", "ff_b1", "ff_w2", "ff_b2",
          "lnf_s", "lnf_b", "head_w")}

    if np.any(f["ln1_b"]) or np.any(f["lnf_b"]) or np.any(f["ff_b2"]):
        raise NotImplementedError("nonzero ln1_b/lnf_b/ff_b2 not supported")

    x0 = f["tok_emb"][tokens] + f["type_emb"][types] + f["pos_emb"][None, :L]
    allowed = _mask_allowed(tokens, attn_mask)            # (B, L, L) [i, j]
    masktr = allowed.transpose(0, 2, 1).astype(np.float32)   # (B, j, i) 0/1

    live = []
    av_live = {c: [] for c in range(IC)}
    partial = []
    trims = {}
    for jt in range(IT):
        for c in range(IC):
            blk = allowed[:, c * 512:(c + 1) * 512,
                          jt * 128:(jt + 1) * 128]
            if blk.any():
                live.append((jt, c))
                av_live[c].append(jt)
                if not blk.all():
                    partial.append((jt, c))
                # columns (queries) with no live key in this block can be
                # skipped entirely when they form a prefix
                live_i = blk.any(axis=(0, 2))
                t0 = int(np.argmax(live_i))
                if not live_i[t0:].all():
                    t0 = 0
                trims[(jt, c)] = t0
    for c in range(IC):
        if av_live[c]:
            # the first AV matmul must cover the full chunk (start=True)
            trims[(av_live[c][0], c)] = 0

    scale = 1.0 / np.sqrt(HD)
    use_gelu_bias = []
    import ml_dtypes
    BF = ml_dtypes.bfloat16

    per_rank_qk = [[] for _ in range(TP)]
    per_rank_v = [[] for _ in range(TP)]
    wo_l, w1_l, gb_l, w2_l = [], [], [], []
    for l in range(NL):
        s1 = f["ln1_s"][l]
        s2, b2ln = f["ln2_s"][l], f["ln2_b"][l]
        for r in range(TP):
            hs = slice(3 * r * HD, 3 * (r + 1) * HD)
            Wq = f["qkv_w"][l][0:D][hs] * scale
            Wk = f["qkv_w"][l][D:2 * D][hs]
            Wv = f["qkv_w"][l][2 * D:3 * D][hs]
            wqk_cat = np.concatenate([Wq, Wk], axis=0)        # (384, 768)
            per_rank_qk[r].append(_sbufify((wqk_cat * s1[None, :]).T, BF))
            WvT = (Wv * s1[None, :]).T                        # (768, 192)
            per_rank_v[r].append(_sbufify(WvT, BF))
        wo_l.append(_sbufify(f["out_w"][l].T, BF))            # (768, 768)
        W1T = (f["ff_w1"][l] * s2[None, :]).T                 # (768, 3072)
        for mt in range(FT):
            w1_l.append(_sbufify(W1T[:, mt * 128:(mt + 1) * 128], BF))
        gbias = f["ff_b1"][l] + f["ff_w1"][l] @ b2ln
        gb_l.append(_sbufify(gbias.reshape(FF, 1)))           # [128, 24]
        W2T = f["ff_w2"][l].T                                 # (3072, 768)
        for mt in range(KT):
            w2_l.append(_sbufify(W2T[:, mt * 128:(mt + 1) * 128], BF))
        use_gelu_bias.append(bool(np.any(gbias != 0.0)))
    wo_all = np.concatenate(wo_l, axis=1)
    w1_all = np.concatenate(w1_l, axis=1)
    gb_all = np.concatenate(gb_l, axis=1)
    w2_all = np.concatenate(w2_l, axis=1)

    # layer-0 LN1 on the host (scale s1 is folded into wqk/wv)
    mu0 = x0.mean(axis=-1, keepdims=True)
    var0 = np.square(x0 - mu0).mean(axis=-1, keepdims=True)
    hln0 = (x0 - mu0) / np.sqrt(var0 + EPS)                   # (B, L, D)

    per_core = []
    for c in range(8):
        b, r = c // 4, c % 4
        vsl = slice(r * VS, (r + 1) * VS)
        x0tb = _sbufify(np.ascontiguousarray(x0[b].T))        # [128, 6*1024]
        im = {}
        im["hln0t"] = _sbufify(np.ascontiguousarray(hln0[b].T), BF)
        im["x0s"] = np.ascontiguousarray(
            x0tb.reshape(128, KT, L)[:, :, r * LS:(r + 1) * LS]
            .reshape(128, KT * LS))
        im["maskt"] = _sbufify(masktr[b], BF)
        im["wqk"] = np.concatenate(per_rank_qk[r], axis=1)
        im["wv"] = np.concatenate(per_rank_v[r], axis=1)
        im["wo"] = wo_all
        im["w1"] = w1_all
        im["gb"] = gb_all
        im["w2"] = w2_all
        Whd = f["head_w"][vsl] * f["lnf_s"][None, :]          # (8000, 768)
        im["wh"] = _sbufify(Whd.T, BF)
        per_core.append(im)
    return per_core, tuple(live), {k: tuple(v) for k, v in av_live.items()}, \
        tuple(use_gelu_bias), tuple(partial), trims


def _run(inputs, trace=False):
    per_core, live, av_live, ugb, partial, trims = _prepare(inputs)
    key = (live, tuple(sorted(av_live.items())), ugb, partial,
           tuple(sorted(trims.items())))
    if key not in _PROG_CACHE:
        _PROG_CACHE[key] = _build(list(live),
                                  {k: list(v) for k, v in av_live.items()},
                                  list(ugb), set(partial), trims)
    nc = _PROG_CACHE[key]
    res = run_bass_kernel_spmd(nc, per_core, core_ids=list(range(8)),
                               trace=trace)
    out = np.empty((B, L, V), dtype=np.float32)
    for c in range(8):
        b, r = c // 4, c % 4
        out[b, :, r * VS:(r + 1) * VS] = \
            res.results[c]["logits"].astype(np.float32)
    return out, res


def kernel(**inputs):
    out, _ = _run(inputs, trace=False)
    return out


# revision 46
# speedup vs baseline: 1.0073x; 1.0073x over previous
"""Trainium2 Bass kernel for a 2-layer causal transformer LM (B=2, L=1024,
D=768, H=12, FF=3072, V=32000) with box-sparse attention mask.

Sharding over 8 NeuronCores: 2-way data parallel over batch x 4-way tensor
parallel within each batch group:
  - attention: 3 heads/core, full-L keys/queries
  - three per-head AllGathers per layer turn head-sharding into
    sequence-sharding (each head's gather overlaps the next head's
    compute), then the out-projection and the full-width FFN run
    sequence-parallel on each core's L/4 position slice
  - the residual stream lives sequence-sharded [768, 256] f32; each layer
    ends with the NEXT LayerNorm (ln1 of l+1, or lnf) computed on the local
    slice and one bf16 AllGather of the post-LN activations; the LN
    statistics matmuls are interleaved into the preceding out-proj / FFN-w2
    loops so the gather triggers as early as possible
  - LM head: V/4 vocab slice per core, bf16 weights streamed in quarters

Device layout: activations transposed [feature, position]; matmuls in
bf16 with f32 PSUM accumulation; LN stats via ones-matmuls on the PE;
softmax without max subtraction (scores are O(1)); the box mask is applied
as a post-exp 0/1 multiply on the vector engine (only for partially-masked
blocks; fully-allowed blocks skip it); softmax denominator via an appended
ones-column in the A@V matmul, broadcast back over partitions with a PE
ones-matmul.
"""
import sys

sys.path.insert(0, "/opt/trn_rl_repo")

from contextlib import ExitStack

import numpy as np
import concourse.bass as bass
import concourse.bacc as bacc
import concourse.mybir as mybir
import concourse.tile as tile
from concourse.bass_utils import run_bass_kernel_spmd

F32 = mybir.dt.float32
F32R = mybir.dt.float32r
BF16 = mybir.dt.bfloat16
AF = mybir.ActivationFunctionType
ALU = mybir.AluOpType

B, L, D, H, HD = 2, 1024, 768, 12, 64
FF, V, NL = 3072, 32000, 2
BOS, SEP, WIN = 1, 2, 512
EPS = 1e-5
TP = 4                      # tensor-parallel group size
NH = H // TP                # heads per core (3)
LS = L // TP                # sequence slice per core (256)
VS = V // TP                # vocab slice per core (8000)
KT = D // 128               # k-tiles over model dim (6)
FT = FF // 128              # k-tiles over ff dim (24)
IT = L // 128               # i/j tiles over positions (8)
IC = L // 512               # 512-wide position chunks (2)
GELU_FUNC = None            # sim-only override hook (AF.Gelu on hardware)


def _mask_allowed(tokens, attn_mask):
    """(B, L, L) boolean allowed[i, j] per reference._box_mask_bias."""
    valid = attn_mask.astype(bool)
    ii = np.arange(L)[:, None]
    jj = np.arange(L)[None, :]
    causal = jj <= ii
    is_sep = (tokens == SEP) & valid
    seg = np.cumsum(is_sep.astype(np.int32), axis=1)
    same_seg = seg[:, :, None] == seg[:, None, :]
    gkey = ((tokens == BOS) & valid) | is_sep
    win = (ii - jj) <= WIN
    return valid[:, None, :] & causal[None] & (
        same_seg | gkey[:, None, :] | win[None])


def _sbufify(w, dtype=np.float32):
    """(K, M) host matrix -> [128, (K/128)*M] SBUF layout; k-tile kt at
    columns [kt*M:(kt+1)*M)."""
    K, M = w.shape
    assert K % 128 == 0
    return np.ascontiguousarray(
        w.reshape(K // 128, 128, M).transpose(1, 0, 2)
        .reshape(128, (K // 128) * M)).astype(dtype)


def _build(live, av_live, use_gelu_bias, partial, trims):
    nc = bacc.Bacc("TRN2", target_bir_lowering=False)

    hln0t = nc.declare_dram_parameter("hln0t", [128, KT * L], BF16,
                                      isOutput=False)
    x0s = nc.declare_dram_parameter("x0s", [128, KT * LS], F32R,
                                    isOutput=False)
    maskt = nc.declare_dram_parameter("maskt", [128, IT * L], BF16,
                                      isOutput=False)
    wqk = nc.declare_dram_parameter("wqk", [128, NL * KT * 384], BF16,
                                    isOutput=False)
    wv = nc.declare_dram_parameter("wv", [128, NL * KT * 192], BF16,
                                   isOutput=False)
    wo = nc.declare_dram_parameter("wo", [128, NL * KT * 768], BF16,
                                   isOutput=False)
    w1 = nc.declare_dram_parameter("w1", [128, NL * FT * (KT * 128)], BF16,
                                   isOutput=False)
    gb = nc.declare_dram_parameter("gb", [128, NL * FT], F32, isOutput=False)
    w2 = nc.declare_dram_parameter("w2", [128, NL * KT * (FT * 128)], BF16,
                                   isOutput=False)
    wh = nc.declare_dram_parameter("wh", [128, KT * VS], BF16, isOutput=False)
    logits = nc.declare_dram_parameter("logits", [L, VS], BF16, isOutput=True)

    groups = [[0, 1, 2, 3], [4, 5, 6, 7]]

    nvc = (VS + 511) // 512
    vchunks = [(i * 512, min(512, VS - i * 512)) for i in range(nvc)]
    quarters = [vchunks[i:i + 4] for i in range(0, nvc, 4)]

    with tile.TileContext(nc) as tc, ExitStack() as ctx:
        const = ctx.enter_context(tc.tile_pool(name="const", bufs=1))
        dram = ctx.enter_context(tc.tile_pool(name="dram", bufs=1,
                                              space="DRAM"))
        resb = ctx.enter_context(tc.tile_pool(name="resb", bufs=1))
        rows = ctx.enter_context(tc.tile_pool(name="rows", bufs=1))
        hw = ctx.enter_context(tc.tile_pool(name="hw", bufs=2))

        # fire a tiny throwaway collective first: the first collective
        # after the runtime's init barrier pays an ~11us cold-start that
        # would otherwise land on the first y-gather
        warm_in = dram.tile([4, 64], BF16, name="warm_in")
        warm_out = dram.tile([TP * 4, 64], BF16, name="warm_out")
        nc.gpsimd.collective_compute(
            "AllGather", ALU.bypass, replica_groups=groups,
            ins=[warm_in.opt()], outs=[warm_out.opt()])

        ones_col = const.tile([128, 1], F32R, name="ones")
        nc.gpsimd.memset(ones_col[:].bitcast(F32), 1.0)
        eps_col = const.tile([1, 1], F32, name="epsc")
        nc.gpsimd.memset(eps_col[:], EPS)
        ones_row = const.tile([1, 128], F32R, name="onesr")
        nc.gpsimd.memset(ones_row[:].bitcast(F32), 1.0)
        seq_off = (nc.partition_id() % TP) * LS

        # persistent state: mask tiles (partial blocks only), post-LN
        # activations, residual slice, and all attention weights (loaded
        # once at startup so no weight DMA ever contends with the
        # latency-critical collectives)
        mtiles = {pc: resb.tile([128, 512], BF16,
                                name=f"msk_{pc[0]}_{pc[1]}")
                  for pc in partial}
        hln = [resb.tile([128, L], BF16, name=f"hln_{k}") for k in range(KT)]
        xs = [resb.tile([128, LS], F32R, name=f"xs_{k}") for k in range(KT)]
        wqk_sb = resb.tile([128, NL * KT * 384], BF16, name="wqkA")
        wv_sb = resb.tile([128, NL * KT * 192], BF16, name="wvA")
        wo_sb = resb.tile([128, NL * KT * 768], BF16, name="woA")

        def load_quarter(vq, gate=None):
            chunks = quarters[vq]
            q0 = chunks[0][0]
            qw = sum(w for _, w in chunks)
            tiles = []
            for k in range(KT):
                t = hw.tile([128, 2048], BF16, name=f"hw_{k}")
                if gate is not None:
                    nc.sync.dma_start(t[0:1, 0:1], gate[0:1, 0:1])
                nc.sync.dma_start(t[:, 0:qw],
                                  wh[:, k * VS + q0:k * VS + q0 + qw])
                tiles.append(t)
            return tiles

        wt_cur = None

        with ExitStack() as body:
            hpool = body.enter_context(tc.tile_pool(name="hpool", bufs=1))
            pm = body.enter_context(tc.tile_pool(name="pm", bufs=6,
                                                 space="PSUM"))
            py = body.enter_context(tc.tile_pool(name="py", bufs=2,
                                                 space="PSUM"))

            def ln_stat(k, src, sx, sxx):
                """Accumulate per-position sum/sumsq of k-tile `src` into
                PSUM rows sx/sxx (start at k==0, stop at k==KT-1)."""
                nc.tensor.matmul(sx[0:1, 0:LS], ones_col[:], src[:, 0:LS],
                                 start=(k == 0), stop=(k == KT - 1))
                xx = hpool.tile([128, LS], F32R, name="lnxx", bufs=2)
                nc.scalar.activation(xx[:, 0:LS], src[:, 0:LS], AF.Square)
                nc.tensor.matmul(sxx[0:1, 0:LS], ones_col[:], xx[:, 0:LS],
                                 start=(k == 0), stop=(k == KT - 1))

            def ln_rows(sx, sxx):
                """Turn the PSUM stats rows into broadcast [128, LS] PSUM
                tiles (rstd, rho=mu*rstd)."""
                mu = rows.tile([1, 512], F32, name="mu")
                nc.vector.tensor_scalar_mul(mu[0:1, 0:LS], sx[0:1, 0:LS],
                                            1.0 / D)
                sq = rows.tile([1, 512], F32, name="sq")
                nc.vector.tensor_mul(sq[0:1, 0:LS], mu[0:1, 0:LS],
                                     mu[0:1, 0:LS])
                dvar = rows.tile([1, 512], F32, name="dvar")
                nc.vector.scalar_tensor_tensor(
                    dvar[0:1, 0:LS], sq[0:1, 0:LS], -float(D),
                    sxx[0:1, 0:LS], op0=ALU.mult, op1=ALU.add)
                rstd = rows.tile([1, 512], F32, name="rstd")
                nc.scalar.activation(rstd[0:1, 0:LS], dvar[0:1, 0:LS],
                                     AF.Sqrt, bias=eps_col[:],
                                     scale=1.0 / D)
                rtmp = rows.tile([1, 512], F32, name="rtmp")
                nc.vector.reciprocal_approx_fast(rtmp[0:1, 0:LS],
                                                 rstd[0:1, 0:LS])
                rstd_row = rows.tile([1, 512], F32R, name="rstd_row")
                with nc.allow_low_precision(reason="f32r feeds bcast"):
                    nc.vector.tensor_copy(rstd_row[0:1, 0:LS],
                                          rtmp[0:1, 0:LS])
                rho_row = rows.tile([1, 512], F32R, name="rho_row")
                nc.vector.tensor_mul(rho_row[0:1, 0:LS], mu[0:1, 0:LS],
                                     rtmp[0:1, 0:LS])
                prs = pm.tile([128, 512], F32, name="pmm")
                nc.tensor.matmul(prs[:, 0:LS], ones_row[:],
                                 rstd_row[0:1, 0:LS], start=True, stop=True)
                pro = pm.tile([128, 512], F32, name="pmm")
                nc.tensor.matmul(pro[:, 0:LS], ones_row[:],
                                 rho_row[0:1, 0:LS], start=True, stop=True)
                return prs, pro

            def ln_apply(src, dst, bcs, c0=0, w=LS):
                prs, pro = bcs
                nc.vector.scalar_tensor_tensor(
                    dst[:, 0:w], src[:, c0:c0 + w], 1.0, prs[:, c0:c0 + w],
                    op0=ALU.bypass, op1=ALU.mult)
                nc.vector.tensor_sub(dst[:, 0:w], dst[:, 0:w],
                                     pro[:, c0:c0 + w])

            # startup loads in priority order: layer-0 QKV weights +
            # layer-0 LN1 (host-precomputed) gate the first matmuls;
            # everything else streams in behind them
            nc.sync.dma_start(wqk_sb[:, 0:KT * 384], wqk[:, 0:KT * 384])
            for k in range(KT):
                nc.sync.dma_start(hln[k][:], hln0t[:, k * L:(k + 1) * L])
            nc.sync.dma_start(wv_sb[:, 0:KT * 192], wv[:, 0:KT * 192])
            nc.sync.dma_start(wqk_sb[:, KT * 384:], wqk[:, KT * 384:])
            nc.sync.dma_start(wv_sb[:, KT * 192:], wv[:, KT * 192:])
            nc.sync.dma_start(wo_sb[:], wo[:, :])
            for k in range(KT):
                nc.sync.dma_start(xs[k][:], x0s[:, k * LS:(k + 1) * LS])
            for (jt, c) in partial:
                nc.sync.dma_start(
                    mtiles[(jt, c)][:],
                    maskt[:, jt * L + c * 512:jt * L + c * 512 + 512])

            for l in range(NL):
                with ExitStack() as lay:
                    wpf = lay.enter_context(tc.tile_pool(name="wpf", bufs=1))
                    W1PF, W2PF = 8, 3

                    def load_w1(mt, gate=None):
                        t = wpf.tile([128, KT * 128], BF16, name="w1c",
                                     bufs=W1PF)
                        if gate is not None:
                            # dummy write creates a WAW dep: the real load
                            # must wait for the gather, so the prefetch
                            # burst cannot steal collective bandwidth
                            nc.sync.dma_start(t[0:1, 0:1], gate[0:1, 0:1])
                        nc.sync.dma_start(
                            t[:], w1[:, (l * FT + mt) * 768:
                                      (l * FT + mt + 1) * 768])
                        return t

                    def load_w2(mt, gate=None):
                        t = wpf.tile([128, FT * 128], BF16, name="w2c",
                                     bufs=W2PF)
                        if gate is not None:
                            nc.sync.dma_start(t[0:1, 0:1], gate[0:1, 0:1])
                        nc.sync.dma_start(
                            t[:], w2[:, (l * KT + mt) * FT * 128:
                                      (l * KT + mt + 1) * FT * 128])
                        return t

                    wqo = l * KT * 384
                    wvo = l * KT * 192
                    woo = l * KT * 768

                    # ================ attention ================
                    with ExitStack() as attn:
                        qkv = attn.enter_context(
                            tc.tile_pool(name="qkv", bufs=1))
                        qp = [qkv.tile([64, L], BF16, name=f"qp{h}")
                              for h in range(NH)]
                        kp = [qkv.tile([64, L], BF16, name=f"kp{h}")
                              for h in range(NH)]
                        vt = [qkv.tile([128, 3 * 65], BF16, name=f"v{j}")
                              for j in range(IT)]
                        yh = [qkv.tile([64, L], BF16, name=f"yh{h}")
                              for h in range(NH)]
                        for j in range(IT):
                            for h in range(NH):
                                nc.gpsimd.memset(
                                    vt[j][:, h * 65 + 64:h * 65 + 65], 1.0)

                        with ExitStack() as s1:
                            # pre-load the exp activation table while the
                            # QKV matmuls run (a table switch costs ~1.3us
                            # on the scalar engine)
                            wrm = rows.tile([1, 1], F32, name="wrm")
                            nc.scalar.activation(wrm[:], eps_col[:], AF.Exp)
                            qk_dest = [(qp[0], qp[1]), (qp[2], kp[0]),
                                       (kp[1], kp[2])]
                            for mt in range(3):
                                for c in range(IC):
                                    p = pm.tile([128, 512], F32, name="pmm")
                                    for k in range(KT):
                                        o = wqo + k * 384 + mt * 128
                                        nc.tensor.matmul(
                                            p[:],
                                            wqk_sb[:, o:o + 128],
                                            hln[k][:, c * 512:(c + 1) * 512],
                                            start=(k == 0),
                                            stop=(k == KT - 1))
                                    t0, t1 = qk_dest[mt]
                                    cs = slice(c * 512, (c + 1) * 512)
                                    nc.vector.tensor_copy(t0[:, cs],
                                                          p[0:64, :])
                                    nc.scalar.activation(t1[:, cs],
                                                         p[64:128, :],
                                                         AF.Copy)
                            for j in range(IT):
                                p = pm.tile([128, 512], F32, name="pmm")
                                for k in range(KT):
                                    o = wvo + k * 192
                                    nc.tensor.matmul(
                                        p[:, 0:192],
                                        hln[k][:, j * 128:(j + 1) * 128],
                                        wv_sb[:, o:o + 192],
                                        start=(k == 0), stop=(k == KT - 1))
                                for h in range(NH):
                                    nc.vector.tensor_copy(
                                        vt[j][:, h * 65:h * 65 + 64],
                                        p[:, h * 64:(h + 1) * 64])

                        # one small AllGather per head: head h's gather is
                        # in flight while head h+1 computes, so only the
                        # last (384KB-recv) gather is exposed on the
                        # critical path.
                        agy_in = [dram.tile([64, L], BF16,
                                            name=f"agy_in{l}_{h}")
                                  for h in range(NH)]
                        agy_out = [dram.tile([TP * 64, L], BF16,
                                             name=f"agy_out{l}_{h}")
                                   for h in range(NH)]
                        with ExitStack() as s2:
                            epool = s2.enter_context(
                                tc.tile_pool(name="epool", bufs=23))
                            spool = s2.enter_context(
                                tc.tile_pool(name="spool", bufs=4))
                            live_co = sorted(live, key=lambda t: (t[1],
                                                                  t[0]))

                            def emit_scores(h):
                                et = {}
                                for (jt, c) in live_co:
                                    t0 = trims[(jt, c)]
                                    cs = slice(t0, 512)
                                    p = pm.tile([128, 512], F32, name="pmm")
                                    nc.tensor.matmul(
                                        p[:, cs],
                                        kp[h][:, jt * 128:(jt + 1) * 128],
                                        qp[h][:, c * 512 + t0:
                                              (c + 1) * 512],
                                        start=True, stop=True)
                                    e = epool.tile([128, 512], BF16,
                                                   name="e")
                                    nc.scalar.activation(e[:, cs], p[:, cs],
                                                         AF.Exp)
                                    if (jt, c) in partial:
                                        nc.vector.tensor_mul(
                                            e[:, cs], e[:, cs],
                                            mtiles[(jt, c)][:, cs])
                                    et[(jt, c)] = e
                                return et

                            def emit_av(h, et):
                                for c in range(IC):
                                    jts = av_live[c]
                                    p = py.tile([128, 512], F32, name="pyy")
                                    for n, jt in enumerate(jts):
                                        t0 = trims[(jt, c)]
                                        nc.tensor.matmul(
                                            p[0:65, t0:512],
                                            vt[jt][:, h * 65:h * 65 + 65],
                                            et[(jt, c)][:, t0:512],
                                            start=(n == 0),
                                            stop=(n == len(jts) - 1),
                                            skip_group_check=True)
                                    den = rows.tile([1, 512], F32,
                                                    name="den")
                                    nc.vector.tensor_copy(den[:],
                                                          p[64:65, :])
                                    rtm = rows.tile([1, 512], F32,
                                                    name="rtm")
                                    nc.vector.reciprocal_approx_fast(
                                        rtm[:], den[:])
                                    rec = rows.tile([1, 512], F32R,
                                                    name="rec")
                                    with nc.allow_low_precision(
                                            reason="softmax denom"):
                                        nc.vector.tensor_copy(rec[:],
                                                              rtm[:])
                                    pb = py.tile([128, 512], F32,
                                                 name="pyy")
                                    nc.tensor.matmul(
                                        pb[0:64, :], ones_row[:, 0:64],
                                        rec[:], start=True, stop=True)
                                    rb = spool.tile([64, 512], BF16,
                                                    name="rb", bufs=2)
                                    # DVE copy: keep the scalar engine free
                                    # for the exp chain
                                    nc.vector.tensor_copy(rb[:],
                                                          pb[0:64, :])
                                    nc.vector.tensor_mul(
                                        yh[h][:, c * 512:(c + 1) * 512],
                                        p[0:64, :], rb[:])
                                # gather this head's y as soon as it is
                                # finished
                                nc.sync.dma_start(agy_in[h][:], yh[h][:])
                                nc.gpsimd.collective_compute(
                                    "AllGather", ALU.bypass,
                                    replica_groups=groups,
                                    ins=[agy_in[h].opt()],
                                    outs=[agy_out[h].opt()])

                            # software-pipelined: the PE computes head
                            # h+1's scores while the scalar engine is
                            # still exponentiating head h's
                            ets = {0: emit_scores(0)}
                            for h in range(NH):
                                if h + 1 < NH:
                                    ets[h + 1] = emit_scores(h + 1)
                                emit_av(h, ets.pop(h))

                        # sequence-parallel out-projection.  Gathered
                        # feature f = 192*s + 64*t + w64 lives at
                        # agy_out[t] row s*64 + w64.
                        with ExitStack() as s3:
                            wpo = s3.enter_context(
                                tc.tile_pool(name="wpo", bufs=1))
                            yrt = [wpo.tile([128, LS], BF16, name=f"yrt_{k}")
                                   for k in range(KT)]

                            f = 0
                            while f < 768:
                                s = f // 192
                                w = f - 192 * s
                                t, w64 = w // 64, w % 64
                                n = min(64 - w64, 128 - f % 128)
                                nc.sync.dma_start(
                                    yrt[f // 128][f % 128:f % 128 + n, :],
                                    agy_out[t][s * 64 + w64:
                                               s * 64 + w64 + n,
                                               bass.ds(seq_off, LS)])
                                f += n
                            # FFN / LM-head weight prefetch is gated on
                            # the last y-gather: an ungated burst steals
                            # HBM bandwidth from the collectives (measured
                            # 15-22GB/s collective bus during the burst)
                            gate = agy_out[NH - 1]
                            w1t = [load_w1(mt, gate) for mt in range(W1PF)]
                            w2t = [load_w2(mt, gate) for mt in range(W2PF)]
                            gb_sb = None
                            if use_gelu_bias[l]:
                                gb_sb = wpf.tile([128, FT], F32, name="gb")
                                nc.sync.dma_start(
                                    gb_sb[:], gb[:, l * FT:(l + 1) * FT])
                            # pre-load the sqrt table for ln2's ln_rows
                            wrm2 = rows.tile([1, 1], F32, name="wrm2")
                            nc.scalar.activation(wrm2[:], eps_col[:],
                                                 AF.Sqrt)
                            sx2 = py.tile([128, 512], F32, name="pyy")
                            sxx2 = py.tile([128, 512], F32, name="pyy")
                            for mt in range(KT):
                                p = pm.tile([128, 512], F32, name="pmm")
                                for k in range(KT):
                                    o = woo + k * 768 + mt * 128
                                    nc.tensor.matmul(
                                        p[:, 0:LS],
                                        wo_sb[:, o:o + 128],
                                        yrt[k][:],
                                        start=(k == 0), stop=(k == KT - 1))
                                nc.vector.tensor_add(
                                    xs[mt][:], xs[mt][:], p[:, 0:LS])
                                ln_stat(mt, xs[mt], sx2, sxx2)

                    # ============ FFN (sequence-parallel) ============
                    with ExitStack() as ffn:
                        mpool = ffn.enter_context(
                            tc.tile_pool(name="mpool", bufs=1))
                        bcs2 = ln_rows(sx2, sxx2)
                        h2s = []
                        for k in range(KT):
                            h = hpool.tile([128, LS], BF16, name=f"h2_{k}")
                            ln_apply(xs[k], h, bcs2)
                            h2s.append(h)
                        mtl = []
                        for mt in range(FT):
                            if mt + W1PF < FT:
                                w1t.append(load_w1(mt + W1PF))
                            p = pm.tile([128, 512], F32, name="pmm")
                            for k in range(KT):
                                nc.tensor.matmul(
                                    p[:, 0:LS],
                                    w1t[mt][:, k * 128:(k + 1) * 128],
                                    h2s[k][:],
                                    start=(k == 0), stop=(k == KT - 1))
                            m = mpool.tile([128, LS], BF16, name=f"m_{mt}")
                            gf = GELU_FUNC or AF.Gelu
                            if gb_sb is not None:
                                nc.scalar.activation(
                                    m[:], p[:, 0:LS], gf,
                                    bias=gb_sb[:, mt:mt + 1])
                            else:
                                nc.scalar.activation(m[:], p[:, 0:LS], gf)
                            mtl.append(m)
                        # pre-load the sqrt table for the boundary LN so
                        # the table switch is off the critical chain
                        wrm3 = rows.tile([1, 1], F32, name="wrm3")
                        nc.scalar.activation(wrm3[:], eps_col[:], AF.Sqrt)
                        sx3 = py.tile([128, 512], F32, name="pyy")
                        sxx3 = py.tile([128, 512], F32, name="pyy")
                        for mt in range(KT):
                            if mt + W2PF < KT:
                                w2t.append(load_w2(mt + W2PF))
                            p = pm.tile([128, 512], F32, name="pmm")
                            for k in range(FT):
                                nc.tensor.matmul(
                                    p[:, 0:LS],
                                    w2t[mt][:, k * 128:(k + 1) * 128],
                                    mtl[k][:],
                                    start=(k == 0), stop=(k == FT - 1))
                            nc.vector.tensor_add(xs[mt][:], xs[mt][:],
                                                 p[:, 0:LS])
                            ln_stat(mt, xs[mt], sx3, sxx3)
                        if l == NL - 1:
                            # LM-head quarter 0 streams in behind the w2
                            # tail; issuing it with the main prefetch
                            # burst jams the in-loop w1/w2 streaming
                            wt_cur = load_quarter(0)

                    # ===== next LN on the local slice + AllGather =====
                    with ExitStack() as nxs:
                        npool = nxs.enter_context(
                            tc.tile_pool(name="npool", bufs=1))
                        bcs3 = ln_rows(sx3, sxx3)
                        # one merged gather: each extra collective costs a
                        # trigger plus up-to-13us of cc-stream delay and
                        # amplifies rank skew, which outweighs the overlap
                        # from split gathers (measured both ways)
                        agh_in = dram.tile([KT * 128, LS], BF16,
                                           name=f"agh_in{l}")
                        agh_out = dram.tile([TP * KT * 128, LS], BF16,
                                            name=f"agh_out{l}")
                        for k in range(KT):
                            nxt = npool.tile([128, LS], BF16,
                                             name=f"nx_{k}")
                            ln_apply(xs[k], nxt, bcs3)
                            nc.sync.dma_start(
                                agh_in[k * 128:(k + 1) * 128, :], nxt[:])
                        nc.gpsimd.collective_compute(
                            "AllGather", ALU.bypass,
                            replica_groups=groups,
                            ins=[agh_in.opt()], outs=[agh_out.opt()])
                        # k-major so hln[k] completes after 4 DMAs
                        for k in range(KT):
                            for q in range(TP):
                                nc.sync.dma_start(
                                    hln[k][:, q * LS:(q + 1) * LS],
                                    agh_out[q * KT * 128 + k * 128:
                                            q * KT * 128 +
                                            (k + 1) * 128, :])

        # ================ LM head ================
        # hln now holds lnf(x) over the full sequence, bf16.
        with ExitStack() as headx:
            ob = headx.enter_context(tc.tile_pool(name="ob", bufs=4))
            ph = headx.enter_context(tc.tile_pool(name="ph", bufs=8,
                                                  space="PSUM"))
            ci = 0
            for vq, chunks in enumerate(quarters):
                q0, qw = chunks[0][0], sum(w for _, w in chunks)
                wt = wt_cur
                if vq + 1 < len(quarters):
                    wt_cur = load_quarter(vq + 1)
                # even i-tiles arrive with the first position-half gather
                for it in (0, 2, 4, 6, 1, 3, 5, 7):
                    ps = [ph.tile([128, 512], F32, name="phh")
                          for _ in range(len(chunks))]
                    # k-outer so the stationary tile (hln it-slice) is
                    # identical for the 4 consecutive matmuls
                    for k in range(KT):
                        for vc, (v0, w) in enumerate(chunks):
                            nc.tensor.matmul(
                                ps[vc][:, 0:w],
                                hln[k][:, it * 128:(it + 1) * 128],
                                wt[k][:, v0 - q0:v0 - q0 + w],
                                start=(k == 0), stop=(k == KT - 1))
                    o = ob.tile([128, 2048], BF16, name="o")
                    for vc, (v0, w) in enumerate(chunks):
                        if ci % 2 == 0:
                            nc.vector.tensor_copy(o[:, v0 - q0:v0 - q0 + w],
                                                  ps[vc][:, 0:w])
                        else:
                            nc.scalar.activation(o[:, v0 - q0:v0 - q0 + w],
                                                 ps[vc][:, 0:w], AF.Copy)
                        ci += 1
                    nc.sync.dma_start(
                        logits[it * 128:(it + 1) * 128, q0:q0 + qw],
                        o[:, 0:qw])
    nc.finalize()
    return nc


_PROG_CACHE = {}


def _prepare(inputs):
    tokens = np.asarray(inputs["tokens"])
    types = np.asarray(inputs["types"])
    attn_mask = np.asarray(inputs["attn_mask"])
    f = {k: np.asarray(inputs[k], dtype=np.float32) for k in
         ("tok_emb", "type_emb", "pos_emb", "qkv_w", "out_w", "ln1_s",
          "ln1_b", "ln2_s", "ln2_b", "ff_w1", "ff_b1", "ff_w2", "ff_b2",
          "lnf_s", "lnf_b", "head_w")}

    if np.any(f["ln1_b"]) or np.any(f["lnf_b"]) or np.any(f["ff_b2"]):
        raise NotImplementedError("nonzero ln1_b/lnf_b/ff_b2 not supported")

    x0 = f["tok_emb"][tokens] + f["type_emb"][types] + f["pos_emb"][None, :L]
    allowed = _mask_allowed(tokens, attn_mask)            # (B, L, L) [i, j]
    masktr = allowed.transpose(0, 2, 1).astype(np.float32)   # (B, j, i) 0/1

    live = []
    av_live = {c: [] for c in range(IC)}
    partial = []
    trims = {}
    for jt in range(IT):
        for c in range(IC):
            blk = allowed[:, c * 512:(c + 1) * 512,
                          jt * 128:(jt + 1) * 128]
            if blk.any():
                live.append((jt, c))
                av_live[c].append(jt)
                if not blk.all():
                    partial.append((jt, c))
                # columns (queries) with no live key in this block can be
                # skipped entirely when they form a prefix
                live_i = blk.any(axis=(0, 2))
                t0 = int(np.argmax(live_i))
                if not live_i[t0:].all():
                    t0 = 0
                trims[(jt, c)] = t0
    for c in range(IC):
        if av_live[c]:
            # the first AV matmul must cover the full chunk (start=True)
            trims[(av_live[c][0], c)] = 0

    scale = 1.0 / np.sqrt(HD)
    use_gelu_bias = []
    import ml_dtypes
    BF = ml_dtypes.bfloat16

    per_rank_qk = [[] for _ in range(TP)]
    per_rank_v = [[] for _ in range(TP)]
    wo_l, w1_l, gb_l, w2_l = [], [], [], []
    for l in range(NL):
        s1 = f["ln1_s"][l]
        s2, b2ln = f["ln2_s"][l], f["ln2_b"][l]
        for r in range(TP):
            hs = slice(3 * r * HD, 3 * (r + 1) * HD)
            Wq = f["qkv_w"][l][0:D][hs] * scale
            Wk = f["qkv_w"][l][D:2 * D][hs]
            Wv = f["qkv_w"][l][2 * D:3 * D][hs]
            wqk_cat = np.concatenate([Wq, Wk], axis=0)        # (384, 768)
            per_rank_qk[r].append(_sbufify((wqk_cat * s1[None, :]).T, BF))
            WvT = (Wv * s1[None, :]).T                        # (768, 192)
            per_rank_v[r].append(_sbufify(WvT, BF))
        wo_l.append(_sbufify(f["out_w"][l].T, BF))            # (768, 768)
        W1T = (f["ff_w1"][l] * s2[None, :]).T                 # (768, 3072)
        for mt in range(FT):
            w1_l.append(_sbufify(W1T[:, mt * 128:(mt + 1) * 128], BF))
        gbias = f["ff_b1"][l] + f["ff_w1"][l] @ b2ln
        gb_l.append(_sbufify(gbias.reshape(FF, 1)))           # [128, 24]
        W2T = f["ff_w2"][l].T                                 # (3072, 768)
        for mt in range(KT):
            w2_l.append(_sbufify(W2T[:, mt * 128:(mt + 1) * 128], BF))
        use_gelu_bias.append(bool(np.any(gbias != 0.0)))
    wo_all = np.concatenate(wo_l, axis=1)
    w1_all = np.concatenate(w1_l, axis=1)
    gb_all = np.concatenate(gb_l, axis=1)
    w2_all = np.concatenate(w2_l, axis=1)

    # layer-0 LN1 on the host (scale s1 is folded into wqk/wv)
    mu0 = x0.mean(axis=-1, keepdims=True)
    var0 = np.square(x0 - mu0).mean(axis=-1, keepdims=True)
    hln0 = (x0 - mu0) / np.sqrt(var0 + EPS)                   # (B, L, D)

    per_core = []
    for c in range(8):
        b, r = c // 4, c % 4
        vsl = slice(r * VS, (r + 1) * VS)
        x0tb = _sbufify(np.ascontiguousarray(x0[b].T))        # [128, 6*1024]
        im = {}
        im["hln0t"] = _sbufify(np.ascontiguousarray(hln0[b].T), BF)
        im["x0s"] = np.ascontiguousarray(
            x0tb.reshape(128, KT, L)[:, :, r * LS:(r + 1) * LS]
            .reshape(128, KT * LS))
        im["maskt"] = _sbufify(masktr[b], BF)
        im["wqk"] = np.concatenate(per_rank_qk[r], axis=1)
        im["wv"] = np.concatenate(per_rank_v[r], axis=1)
        im["wo"] = wo_all
        im["w1"] = w1_all
        im["gb"] = gb_all
        im["w2"] = w2_all
        Whd = f["head_w"][vsl] * f["lnf_s"][None, :]          # (8000, 768)
        im["wh"] = _sbufify(Whd.T, BF)
        per_core.append(im)
    return per_core, tuple(live), {k: tuple(v) for k, v in av_live.items()}, \
        tuple(use_gelu_bias), tuple(partial), trims


def _run(inputs, trace=False):
    per_core, live, av_live, ugb, partial, trims = _prepare(inputs)
    key = (live, tuple(sorted(av_live.items())), ugb, partial,
           tuple(sorted(trims.items())))
    if key not in _PROG_CACHE:
        _PROG_CACHE[key] = _build(list(live),
                                  {k: list(v) for k, v in av_live.items()},
                                  list(ugb), set(partial), trims)
    nc = _PROG_CACHE[key]
    res = run_bass_kernel_spmd(nc, per_core, core_ids=list(range(8)),
                               trace=trace)
    out = np.empty((B, L, V), dtype=np.float32)
    for c in range(8):
        b, r = c // 4, c % 4
        out[b, :, r * VS:(r + 1) * VS] = \
            res.results[c]["logits"].astype(np.float32)
    return out, res


def kernel(**inputs):
    out, _ = _run(inputs, trace=False)
    return out
